# revision 16
# baseline (speedup 1.0000x reference)
# Multi-head attention block (QKV proj + per-head q/k layernorm + softmax
# attention + output proj) on 8 Trainium2 NeuronCores.
#
# Sharding: data-parallel over (batch, query-half). Core c handles batch
# c//2, query tokens [ (c%2)*1024, (c%2+1)*1024 ). Each core computes K/V
# for its batch's full 2048 tokens; no cross-core communication.
#
# Per-core dataflow (v2, engine-balanced + software-pipelined):
#   Phase A (QKV): host-pre-blocked xT / W tiles -> PE matmuls (N=512
#   chunks, q/k bias via K=1 ones-row matmul) -> grouped bn_stats (one per
#   512-wide tile) + per-group batched stats chain -> layernorm apply as two
#   broadcast tensor_tensor ops -> one wide XBAR DMA-transpose per tile into
#   feature-major qT/kT. V evicted via DVE add of a broadcast bias row.
#   Phase C (attention): software-pipelined unit stream (scores of unit u+1
#   and attn@V of unit u emitted around exp(u)) so ACT runs exp back-to-back
#   while PE stays dense; softmax normalizer Z rides as PSUM row 64 via a
#   ones-column in V; py evicted RAW + 1/Z kept per-j, normalization applied
#   later via one broadcast multiply per head-pair (Z broadcast via DRAM
#   bounce on the gpsimd queue, off the critical path).
#   Phase D: output projection from feature-major yT, bias via ones-row MM.
import contextlib

import numpy as np
import ml_dtypes

B, T, E = 4, 2048, 1024
H, D = 16, 64
P = 128
EPS = 1e-5
SCALE = 0.125  # 1/sqrt(D)
TQ = T // 2          # query tokens per core
KB = E // P          # contraction blocks
MQ = TQ // P         # query token tiles
MKV = T // P         # kv token tiles
FT = E // P          # feature tiles (qT/kT/yT)
NCORES = 8
HC = 8               # heads per 512-wide feature chunk
NCH = 512
NG = 2               # feature chunks

_BUILT = {}
_last_in_maps = None


def _build_real(affine: bool):
    import concourse.bass as bass
    import concourse.bacc as bacc
    import concourse.tile as tile
    from concourse import mybir

    f32 = mybir.dt.float32
    bf16 = mybir.dt.bfloat16
    AF = mybir.ActivationFunctionType
    OP = mybir.AluOpType

    nc = bacc.Bacc("TRN2", target_bir_lowering=False)
    xq_blk = nc.declare_dram_parameter("xq_blk", [MQ, P, KB, P], bf16, isOutput=False)
    xkv_blk = nc.declare_dram_parameter("xkv_blk", [MKV, P, KB, P], bf16, isOutput=False)
    Wblk = nc.declare_dram_parameter("Wblk", [3 * NG, P, KB, NCH], bf16, isOutput=False)
    bqkv = nc.declare_dram_parameter("bqkv", [3 * E], bf16, isOutput=False)
    q_gamma = nc.declare_dram_parameter("q_gamma", [D], f32, isOutput=False)
    q_beta = nc.declare_dram_parameter("q_beta", [D], f32, isOutput=False)
    k_gamma = nc.declare_dram_parameter("k_gamma", [D], f32, isOutput=False)
    k_beta = nc.declare_dram_parameter("k_beta", [D], f32, isOutput=False)
    Wpblk = nc.declare_dram_parameter("Wpblk", [P, KB, E], bf16, isOutput=False)
    bproj = nc.declare_dram_parameter("bproj", [E], bf16, isOutput=False)
    out = nc.declare_dram_parameter("out", [TQ, E], f32, isOutput=True)

    def bcast(dst, tensor, off, nparts, n):
        ap = bass.AP(tensor=tensor, offset=off, ap=[[0, nparts], [1, n]])
        nc.gpsimd.dma_start(out=dst, in_=ap)

    with tile.TileContext(nc) as tc, contextlib.ExitStack() as top:
        const = top.enter_context(tc.tile_pool(name="const", bufs=1))
        persist = top.enter_context(tc.tile_pool(name="persist", bufs=1))
        dr = top.enter_context(tc.tile_pool(name="dr", bufs=1, space="DRAM"))
        ps = top.enter_context(tc.tile_pool(name="ps", bufs=1, space="PSUM"))

        ones = const.tile([P, P], bf16)
        nc.vector.memset(ones[:], 1.0)
        eps4k = const.tile([P, 1], f32)
        nc.vector.memset(eps4k[:], 4096.0 * EPS)
        bqk_row = const.tile([1, 2 * E], bf16)
        nc.sync.dma_start(out=bqk_row[:], in_=bqkv[0:2 * E])
        bproj_row = const.tile([1, E], bf16)
        nc.sync.dma_start(out=bproj_row[:], in_=bproj[:])
        bias_vb = const.tile([P, E], bf16)
        bcast(bias_vb[:], bqkv, 2 * E, P, E)
        wp_all = const.tile([P, KB, E], bf16)
        nc.sync.dma_start(out=wp_all[:], in_=Wpblk[:])
        if affine:
            gq_t = const.tile([P, D], f32)
            bq_t = const.tile([P, D], f32)
            gk_t = const.tile([P, D], f32)
            bk_t = const.tile([P, D], f32)
            bcast(gq_t[:], q_gamma, 0, P, D)
            bcast(bq_t[:], q_beta, 0, P, D)
            bcast(gk_t[:], k_gamma, 0, P, D)
            bcast(bk_t[:], k_beta, 0, P, D)

        va_all = persist.tile([P, MKV, H, D + 1], bf16)
        nc.vector.memset(va_all[:, :, :, D], 1.0)
        qT_all = persist.tile([P, FT, TQ], bf16)
        kT_all = persist.tile([P, FT, T], bf16)
        yT_raw = persist.tile([P, FT, TQ], bf16)
        yT_all = persist.tile([P, FT, TQ], bf16)
        z_dram = dr.tile([2, FT, TQ], f32)

        def bc_mid(t2d, nmid, ninner):
            # [P, ninner] AP viewed as [P, nmid(bcast), ninner]
            return bass.AP(tensor=t2d.tensor, offset=t2d.offset,
                           ap=[t2d.ap[0], [0, nmid], [1, ninner]])

        def perm_dh(t2d):
            # [P, 512] viewed as [P, d(64), h(8)] (d outer stride 1)
            return bass.AP(tensor=t2d.tensor, offset=t2d.offset,
                           ap=[t2d.ap[0], [1, D], [D, HC]])

        # ---------------- phase A: QKV + LN + transposes ----------------
        with contextlib.ExitStack() as pa:
            wchp = pa.enter_context(tc.tile_pool(name="wchp", bufs=1))
            xs = pa.enter_context(tc.tile_pool(name="xs", bufs=1))
            rawp = pa.enter_context(tc.tile_pool(name="rawp", bufs=1))
            stp = pa.enter_context(tc.tile_pool(name="stp", bufs=1))
            ntp = pa.enter_context(tc.tile_pool(name="ntp", bufs=1))

            def load_wch(kind, g):
                wch = wchp.tile([P, KB, NCH], bf16, name=f"w_{kind}{g}",
                                tag="wch", bufs=3)
                idx = {"q": 0, "k": 2, "v": 4}[kind] + g
                nc.sync.dma_start(out=wch[:], in_=Wblk[idx])
                return wch

            def qkv_mm(xm, wch, bias_off, name):
                pt = ps.tile([P, NCH], f32, name=name, tag="py", bufs=2)
                for kb in range(KB):
                    nc.tensor.matmul(pt[:], xm[:, kb, :], wch[:, kb, :],
                                     start=(kb == 0), stop=(bias_off is None
                                                            and kb == KB - 1))
                if bias_off is not None:
                    nc.tensor.matmul(pt[:], ones[0:1, :],
                                     bqk_row[0:1, bias_off:bias_off + NCH],
                                     start=False, stop=True)
                return pt

            def chain(S, Q, nt_, uniq):
                # S=sum(x), Q=sum(x^2) per [P, nt, HC] ->
                #   rstd = 64/sqrt(64Q - S^2 + 4096 eps), nb = -S/sqrt(...)
                w = lambda nm: stp.tile([P, nt_, HC], f32,
                                        name=f"{uniq}_{nm}",
                                        tag=f"ch_{nm}", bufs=1)
                t1, t2, t3, sq, r, sr = (w(n) for n in
                                         ("t1", "t2", "t3", "sq", "r", "sr"))
                r8 = stp.tile([P, nt_, HC], f32, name=f"{uniq}_r8",
                              tag="ch_r8", bufs=1)
                nb = stp.tile([P, nt_, HC], f32, name=f"{uniq}_nb",
                              tag="ch_nb", bufs=1)
                nc.vector.tensor_tensor(out=t1[:], in0=S[:], in1=S[:], op=OP.mult)
                nc.scalar.mul(t2[:], Q[:], 64.0)
                nc.vector.tensor_tensor(out=t3[:], in0=t2[:], in1=t1[:],
                                        op=OP.subtract)
                nc.scalar.activation(out=sq[:], in_=t3[:], func=AF.Sqrt,
                                     bias=eps4k[:])
                nc.vector.reciprocal(out=r[:], in_=sq[:])
                nc.scalar.mul(r8[:], r[:], 64.0)
                nc.vector.tensor_tensor(out=sr[:], in0=S[:], in1=r[:], op=OP.mult)
                nc.vector.tensor_scalar(out=nb[:], in0=sr[:], scalar1=-1.0,
                                        scalar2=None, op0=OP.mult)
                return r8, nb

            def apply_ln(raw, r8, nb, m, dst_t, gt, bt):
                # dst = raw * r8[:,m,:] + nb[:,m,:]  (broadcast along D)
                tmp = ntp.tile([P, NCH], bf16, tag="tmp", bufs=3)
                nt = ntp.tile([P, NCH], bf16, tag="nt", bufs=3)
                rm = r8[:, m, :]
                nm = nb[:, m, :]
                r_b = bass.AP(tensor=rm.tensor, offset=rm.offset,
                              ap=[rm.ap[0], [0, D], [1, HC]])
                n_b = bass.AP(tensor=nm.tensor, offset=nm.offset,
                              ap=[nm.ap[0], [0, D], [1, HC]])
                nc.vector.tensor_tensor(out=perm_dh(tmp[:]), in0=perm_dh(raw[:]),
                                        in1=r_b, op=OP.mult)
                if affine:
                    nt2 = ntp.tile([P, NCH], bf16, tag="nt2", bufs=3)
                    nc.vector.tensor_tensor(out=perm_dh(nt2[:]),
                                            in0=perm_dh(tmp[:]), in1=n_b,
                                            op=OP.add)
                    n3 = nt2[:].rearrange("p (h d) -> p h d", h=HC)
                    t3 = nt[:].rearrange("p (h d) -> p h d", h=HC)
                    nc.vector.tensor_tensor(out=t3, in0=n3,
                                            in1=bc_mid(gt[:], HC, D), op=OP.mult)
                    nc.vector.tensor_tensor(out=t3, in0=t3,
                                            in1=bc_mid(bt[:], HC, D), op=OP.add)
                else:
                    nc.vector.tensor_tensor(out=perm_dh(nt[:]), in0=perm_dh(tmp[:]),
                                            in1=n_b, op=OP.add)
                return nt

            for g in range(NG):
                wq = load_wch("q", g)
                wk = load_wch("k", g)
                wv = load_wch("v", g)

                def group8(kind, g, m0, wch, bias_off, xblk, dstT, gt, bt):
                    S = stp.tile([P, MQ, HC], f32, name=f"S_{kind}{g}_{m0}",
                                 tag="Ssum", bufs=2)
                    Q = stp.tile([P, MQ, HC], f32, name=f"Q_{kind}{g}_{m0}",
                                 tag="Qsum", bufs=2)
                    raws = []
                    for i in range(MQ):
                        m = m0 + i
                        xm = xs.tile([P, KB, P], bf16,
                                     name=f"x{kind}{g}_{m}", tag="x", bufs=2)
                        nc.sync.dma_start(out=xm[:], in_=xblk[m])
                        pt = qkv_mm(xm, wch, bias_off, f"pt{kind}{g}_{m}")
                        pt3 = pt[:].rearrange("p (h d) -> p h d", h=HC)
                        nc.vector.tensor_reduce(out=S[:, i, :], in_=pt3,
                                                axis=mybir.AxisListType.X,
                                                op=OP.add)
                        sqv = ntp.tile([P, NCH], f32, name=f"sq{kind}{g}_{m}",
                                       tag="sqv", bufs=2)
                        nc.scalar.activation(out=sqv[:], in_=pt[:],
                                             func=AF.Square)
                        nc.vector.tensor_reduce(
                            out=Q[:, i, :],
                            in_=sqv[:].rearrange("p (h d) -> p h d", h=HC),
                            axis=mybir.AxisListType.X, op=OP.add)
                        raw = rawp.tile([P, NCH], bf16, name=f"r{kind}{g}_{m}",
                                        tag="raw8", bufs=MQ)
                        nc.vector.tensor_copy(out=raw[:], in_=pt[:])
                        raws.append(raw)
                        if kind == "k":
                            pv = qkv_mm(xm, wv, None, f"ptv{g}_{m}")
                            nc.vector.tensor_tensor(
                                out=va_all[:, m, g * HC:(g + 1) * HC, 0:D],
                                in0=pv[:].rearrange("p (h d) -> p h d", h=HC),
                                in1=bias_vb[:, g * NCH:(g + 1) * NCH].rearrange(
                                    "p (h d) -> p h d", h=HC),
                                op=OP.add)
                    r8, nb = chain(S, Q, MQ, f"{kind}{g}_{m0}")
                    for i in range(MQ):
                        nt = apply_ln(raws[i], r8, nb, i, None, gt, bt)
                        nc.sync.dma_start_transpose(
                            out=dstT[:, 4 * g:4 * g + 4,
                                     (m0 + i) * P:(m0 + i + 1) * P],
                            in_=nt[:])

                group8("q", g, 0, wq, g * NCH, xq_blk, qT_all,
                       gq_t if affine else None, bq_t if affine else None)
                for m0 in (0, MQ):
                    group8("k", g, m0, wk, E + g * NCH, xkv_blk, kT_all,
                           gk_t if affine else None, bk_t if affine else None)

        # -------- phase C: attention, software-pipelined unit stream --------
        with contextlib.ExitStack() as pc:
            pp = pc.enter_context(tc.tile_pool(name="pp", bufs=1))
            zp = pc.enter_context(tc.tile_pool(name="zp", bufs=1))

            NU = (H // 2) * MKV
            py_tiles = {}

            def emit_scores(u):
                j, tkb = divmod(u, MKV)
                sA = ps.tile([P, TQ], f32, name=f"sA_{u}", tag="scr2", bufs=2)
                sB = ps.tile([P, TQ], f32, name=f"sB_{u}", tag="scr2", bufs=2)
                for nk in range(TQ // NCH):
                    nsl = slice(nk * NCH, (nk + 1) * NCH)
                    nc.tensor.matmul(
                        sA[:, nsl],
                        kT_all[0:D, j, tkb * P:(tkb + 1) * P],
                        qT_all[0:D, j, nsl],
                        start=True, stop=True, tile_position=(0, 0))
                    nc.tensor.matmul(
                        sB[:, nsl],
                        kT_all[D:P, j, tkb * P:(tkb + 1) * P],
                        qT_all[D:P, j, nsl],
                        start=True, stop=True, tile_position=(64, 0))
                return sA, sB

            def emit_exp(u, s):
                sA, sB = s
                pA = pp.tile([P, TQ], bf16, name=f"pA_{u}", tag="p_bf", bufs=4)
                pB = pp.tile([P, TQ], bf16, name=f"pB_{u}", tag="p_bf", bufs=4)
                nc.scalar.activation(out=pA[:], in_=sA[:], func=AF.Exp,
                                     scale=SCALE)
                nc.scalar.activation(out=pB[:], in_=sB[:], func=AF.Exp,
                                     scale=SCALE)
                return pA, pB

            def emit_av(u, p):
                j, tkb = divmod(u, MKV)
                pA, pB = p
                if tkb == 0:
                    py_tiles[j] = (
                        ps.tile([P, TQ], f32, name=f"pyA_{j}", tag="py", bufs=2),
                        ps.tile([P, TQ], f32, name=f"pyB_{j}", tag="py", bufs=2))
                pyA, pyB = py_tiles[j]
                hA, hB = 2 * j, 2 * j + 1
                for nk in range(TQ // NCH):
                    nsl = slice(nk * NCH, (nk + 1) * NCH)
                    nc.tensor.matmul(pyA[0:D + 1, nsl], va_all[:, tkb, hA, :],
                                     pA[:, nsl],
                                     start=(tkb == 0), stop=(tkb == MKV - 1))
                    nc.tensor.matmul(pyB[0:D + 1, nsl], va_all[:, tkb, hB, :],
                                     pB[:, nsl],
                                     start=(tkb == 0), stop=(tkb == MKV - 1))

            def emit_tail(j):
                pyA, pyB = py_tiles.pop(j)
                zrA = zp.tile([1, TQ], f32, name=f"zrA_{j}", tag="zrec", bufs=4)
                zrB = zp.tile([1, TQ], f32, name=f"zrB_{j}", tag="zrec", bufs=4)
                nc.vector.reciprocal(out=zrA[:], in_=pyA[D:D + 1, :])
                nc.vector.reciprocal(out=zrB[:], in_=pyB[D:D + 1, :])
                nc.vector.tensor_copy(out=yT_raw[0:D, j, :], in_=pyA[0:D, :])
                nc.vector.tensor_copy(out=yT_raw[D:P, j, :], in_=pyB[0:D, :])
                nc.sync.dma_start(out=z_dram[0, j, :], in_=zrA[:])
                nc.sync.dma_start(out=z_dram[1, j, :], in_=zrB[:])

            def emit_norm(j):
                zrep = zp.tile([P, TQ], bf16, name=f"zrep_{j}", tag="zrep",
                               bufs=2)
                zt = z_dram[:]
                for hh in range(2):
                    src = bass.AP(tensor=zt.tensor,
                                  offset=zt.offset + (hh * FT + j) * TQ,
                                  ap=[[0, D], [1, TQ]])
                    nc.gpsimd.dma_start(out=zrep[hh * D:(hh + 1) * D, :],
                                        in_=src)
                nc.vector.tensor_tensor(out=yT_all[:, j, :],
                                        in0=yT_raw[:, j, :], in1=zrep[:],
                                        op=OP.mult)

            s_cur = emit_scores(0)
            av_p = None
            for u in range(NU):
                p_cur = emit_exp(u, s_cur)
                if av_p is not None:
                    emit_av(u - 1, av_p)
                    if (u - 1) % MKV == MKV - 1:
                        jj = (u - 1) // MKV
                        emit_tail(jj)
                        emit_norm(jj)
                if u + 1 < NU:
                    s_cur = emit_scores(u + 1)
                av_p = p_cur
            emit_av(NU - 1, av_p)
            emit_tail(H // 2 - 1)
            emit_norm(H // 2 - 1)

        # ---- phase D: output projection ----
        with contextlib.ExitStack() as pd:
            dwork = pd.enter_context(tc.tile_pool(name="dwork", bufs=1))
            for m in range(MQ):
                po = ps.tile([P, E], f32, name=f"po_{m}", tag="py", bufs=2)
                for nk in range(E // NCH):
                    nsl = slice(nk * NCH, (nk + 1) * NCH)
                    for kb in range(KB):
                        nc.tensor.matmul(po[:, nsl],
                                         yT_all[:, kb, m * P:(m + 1) * P],
                                         wp_all[:, kb, nsl],
                                         start=(kb == 0), stop=False)
                    nc.tensor.matmul(po[:, nsl], ones[0:1, :],
                                     bproj_row[0:1, nsl], start=False, stop=True)
                out_sb = dwork.tile([P, E], f32, tag="out_sb", bufs=2)
                nc.scalar.activation(out=out_sb[:], in_=po[:], func=AF.Identity)
                nc.sync.dma_start(out=out[m * P:(m + 1) * P, :], in_=out_sb[:])

    nc.finalize()
    return nc


def _get_nc(affine: bool):
    key = bool(affine)
    if key not in _BUILT:
        _BUILT[key] = _build_real(key)
    return _BUILT[key]


def kernel(x, Wqkv, bqkv, q_gamma, q_beta, k_gamma, k_beta, Wproj, bproj):
    from concourse.bass_utils import run_bass_kernel_spmd

    x = np.asarray(x, dtype=np.float32)
    Wqkv = np.asarray(Wqkv, dtype=np.float32)
    bqkv = np.asarray(bqkv, dtype=np.float32)
    Wproj = np.asarray(Wproj, dtype=np.float32)
    bproj = np.asarray(bproj, dtype=np.float32)
    q_gamma = np.asarray(q_gamma, dtype=np.float32)
    q_beta = np.asarray(q_beta, dtype=np.float32)
    k_gamma = np.asarray(k_gamma, dtype=np.float32)
    k_beta = np.asarray(k_beta, dtype=np.float32)

    affine = not (np.all(q_gamma == 1.0) and np.all(q_beta == 0.0)
                  and np.all(k_gamma == 1.0) and np.all(k_beta == 0.0))
    nc = _get_nc(affine)

    bf = ml_dtypes.bfloat16
    # Wblk[kind*2+g, p, kb, f] = Wqkv[kb*128+p, kind*E + g*512 + f]
    Wq3 = Wqkv.astype(bf).reshape(KB, P, 3, NG, NCH)  # kb, p, kind, g, f
    Wblk = np.ascontiguousarray(Wq3.transpose(2, 3, 1, 0, 4).reshape(
        3 * NG, P, KB, NCH))
    # Wpblk[p, kb, f] = Wproj[kb*128+p, f]
    Wpblk = np.ascontiguousarray(
        Wproj.astype(bf).reshape(KB, P, E).transpose(1, 0, 2))
    bqkv_b = bqkv.astype(bf)
    bproj_b = bproj.astype(bf)

    in_maps = []
    for c in range(NCORES):
        b, half = divmod(c, 2)
        xb = x[b].astype(bf)                       # [T, E]
        # xkv_blk[m, p, kb, t] = x[b][m*128+t, kb*128+p]
        xkv = np.ascontiguousarray(
            xb.reshape(MKV, P, KB, P).transpose(0, 3, 2, 1))
        xq = np.ascontiguousarray(xkv[half * MQ:(half + 1) * MQ])
        in_maps.append({
            "xq_blk": xq, "xkv_blk": xkv,
            "Wblk": Wblk, "bqkv": bqkv_b,
            "q_gamma": q_gamma, "q_beta": q_beta,
            "k_gamma": k_gamma, "k_beta": k_beta,
            "Wpblk": Wpblk, "bproj": bproj_b,
        })

    global _last_in_maps
    _last_in_maps = in_maps
    res = run_bass_kernel_spmd(nc, in_maps, core_ids=list(range(NCORES)))
    y = np.empty((B, T, E), dtype=np.float32)
    for c in range(NCORES):
        b, half = divmod(c, 2)
        y[b, half * TQ:(half + 1) * TQ, :] = res.results[c]["out"]
    return y


# revision 23
# speedup vs baseline: 1.3561x; 1.3561x over previous
# Multi-head attention block (QKV proj + per-head q/k layernorm + softmax
# attention + output proj) on 8 Trainium2 NeuronCores.
#
# Sharding: data-parallel over (batch, query-half). Core c handles batch
# c//2, query tokens [ (c%2)*1024, (c%2+1)*1024 ). Each core computes K/V
# for its batch's full 2048 tokens; no cross-core communication.
#
# Per-core dataflow (v2, engine-balanced + software-pipelined):
#   Phase A (QKV): host-pre-blocked xT / W tiles -> PE matmuls (N=512
#   chunks, q/k bias via K=1 ones-row matmul) -> grouped bn_stats (one per
#   512-wide tile) + per-group batched stats chain -> layernorm apply as two
#   broadcast tensor_tensor ops -> one wide XBAR DMA-transpose per tile into
#   feature-major qT/kT. V evicted via DVE add of a broadcast bias row.
#   Phase C (attention): software-pipelined unit stream (scores of unit u+1
#   and attn@V of unit u emitted around exp(u)) so ACT runs exp back-to-back
#   while PE stays dense; softmax normalizer Z rides as PSUM row 64 via a
#   ones-column in V; py evicted RAW + 1/Z kept per-j, normalization applied
#   later via one broadcast multiply per head-pair (Z broadcast via DRAM
#   bounce on the gpsimd queue, off the critical path).
#   Phase D: output projection from feature-major yT, bias via ones-row MM.
import contextlib

import numpy as np
import ml_dtypes

B, T, E = 4, 2048, 1024
H, D = 16, 64
P = 128
EPS = 1e-5
SCALE = 0.125  # 1/sqrt(D)
TQ = T // 2          # query tokens per core
KB = E // P          # contraction blocks
MQ = TQ // P         # query token tiles
MKV = T // P         # kv token tiles
FT = E // P          # feature tiles (qT/kT/yT)
NCORES = 8
HC = 8               # heads per 512-wide feature chunk
NCH = 512
NG = 2               # feature chunks

_BUILT = {}
_last_in_maps = None


def _build_real(affine: bool):
    import concourse.bass as bass
    import concourse.bacc as bacc
    import concourse.tile as tile
    from concourse import mybir

    f32 = mybir.dt.float32
    bf16 = mybir.dt.bfloat16
    AF = mybir.ActivationFunctionType
    OP = mybir.AluOpType

    nc = bacc.Bacc("TRN2", target_bir_lowering=False)
    xq_blk = nc.declare_dram_parameter("xq_blk", [MQ, P, KB, P], bf16, isOutput=False)
    xkv_blk = nc.declare_dram_parameter("xkv_blk", [MKV, P, KB, P], bf16, isOutput=False)
    Wblk = nc.declare_dram_parameter("Wblk", [3 * NG, P, KB, NCH], bf16, isOutput=False)
    bqkv = nc.declare_dram_parameter("bqkv", [3 * E], bf16, isOutput=False)
    q_gamma = nc.declare_dram_parameter("q_gamma", [D], f32, isOutput=False)
    q_beta = nc.declare_dram_parameter("q_beta", [D], f32, isOutput=False)
    k_gamma = nc.declare_dram_parameter("k_gamma", [D], f32, isOutput=False)
    k_beta = nc.declare_dram_parameter("k_beta", [D], f32, isOutput=False)
    Wpblk = nc.declare_dram_parameter("Wpblk", [P, KB, E], bf16, isOutput=False)
    bproj = nc.declare_dram_parameter("bproj", [E], bf16, isOutput=False)
    out = nc.declare_dram_parameter("out", [TQ, E], f32, isOutput=True)

    def bcast(dst, tensor, off, nparts, n):
        ap = bass.AP(tensor=tensor, offset=off, ap=[[0, nparts], [1, n]])
        nc.gpsimd.dma_start(out=dst, in_=ap)

    with tile.TileContext(nc) as tc, contextlib.ExitStack() as top:
        const = top.enter_context(tc.tile_pool(name="const", bufs=1))
        persist = top.enter_context(tc.tile_pool(name="persist", bufs=1))
        dr = top.enter_context(tc.tile_pool(name="dr", bufs=1, space="DRAM"))
        ps = top.enter_context(tc.tile_pool(name="ps", bufs=1, space="PSUM"))

        ones = const.tile([P, P], bf16)
        nc.vector.memset(ones[:], 1.0)
        eps4k = const.tile([P, 1], f32)
        nc.vector.memset(eps4k[:], 4096.0 * EPS)
        bqk_row = const.tile([1, 3 * E], bf16)
        nc.sync.dma_start(out=bqk_row[:], in_=bqkv[:])
        bproj_row = const.tile([1, E], bf16)
        nc.sync.dma_start(out=bproj_row[:], in_=bproj[:])
        wp_all = const.tile([P, KB, E], bf16)
        nc.sync.dma_start(out=wp_all[:], in_=Wpblk[:])
        if affine:
            gq_t = const.tile([P, D], f32)
            bq_t = const.tile([P, D], f32)
            gk_t = const.tile([P, D], f32)
            bk_t = const.tile([P, D], f32)
            bcast(gq_t[:], q_gamma, 0, P, D)
            bcast(bq_t[:], q_beta, 0, P, D)
            bcast(gk_t[:], k_gamma, 0, P, D)
            bcast(bk_t[:], k_beta, 0, P, D)

        va_all = persist.tile([P, MKV, H, D + 1], bf16)
        nc.vector.memset(va_all[:, :, :, D], 1.0)
        qT_all = persist.tile([P, FT, TQ], bf16)
        kT_all = persist.tile([P, FT, T], bf16)
        yT_raw = persist.tile([P, FT, TQ], bf16)
        yT_all = persist.tile([P, FT, TQ], bf16)
        z_sb = persist.tile([H, TQ], f32)
        z_dram = dr.tile([H, TQ], f32)

        def bc_mid(t2d, nmid, ninner):
            # [P, ninner] AP viewed as [P, nmid(bcast), ninner]
            return bass.AP(tensor=t2d.tensor, offset=t2d.offset,
                           ap=[t2d.ap[0], [0, nmid], [1, ninner]])

        def bc_inner(t2d):
            # [P, HC] AP viewed as [P, h(8), d(64 bcast)]
            return bass.AP(tensor=t2d.tensor, offset=t2d.offset,
                           ap=[t2d.ap[0], [1, HC], [0, D]])

        # ---------------- phase A: QKV + LN + transposes ----------------
        with contextlib.ExitStack() as pa:
            wchp = pa.enter_context(tc.tile_pool(name="wchp", bufs=1))
            xs = pa.enter_context(tc.tile_pool(name="xs", bufs=1))
            rawp = pa.enter_context(tc.tile_pool(name="rawp", bufs=1))
            stp = pa.enter_context(tc.tile_pool(name="stp", bufs=1))
            ntp = pa.enter_context(tc.tile_pool(name="ntp", bufs=1))

            def load_wch(kind, g):
                wch = wchp.tile([P, KB, NCH], bf16, name=f"w_{kind}{g}",
                                tag="wch", bufs=3)
                idx = {"q": 0, "k": 2, "v": 4}[kind] + g
                nc.sync.dma_start(out=wch[:], in_=Wblk[idx])
                return wch

            def qkv_mm(xm, wch, bias_off, name):
                pt = ps.tile([P, NCH], f32, name=name, tag="py", bufs=2)
                for kb in range(KB):
                    nc.tensor.matmul(pt[:], xm[:, kb, :], wch[:, kb, :],
                                     start=(kb == 0), stop=(bias_off is None
                                                            and kb == KB - 1))
                if bias_off is not None:
                    nc.tensor.matmul(pt[:], ones[0:1, :],
                                     bqk_row[0:1, bias_off:bias_off + NCH],
                                     start=False, stop=True)
                return pt

            def chain(S, Q, nt_, uniq):
                # S=sum(x), Q=sum(x^2) per [P, nt, HC] ->
                #   rstd = 64/sqrt(64Q - S^2 + 4096 eps), nb = -S/sqrt(...)
                w = lambda nm: stp.tile([P, nt_, HC], f32,
                                        name=f"{uniq}_{nm}",
                                        tag=f"ch_{nm}", bufs=1)
                t1, t2, t3, sq, r, sr = (w(n) for n in
                                         ("t1", "t2", "t3", "sq", "r", "sr"))
                r8 = stp.tile([P, nt_, HC], f32, name=f"{uniq}_r8",
                              tag="ch_r8", bufs=1)
                nb = stp.tile([P, nt_, HC], f32, name=f"{uniq}_nb",
                              tag="ch_nb", bufs=1)
                nc.vector.tensor_tensor(out=t1[:], in0=S[:], in1=S[:], op=OP.mult)
                nc.scalar.mul(t2[:], Q[:], 64.0)
                nc.vector.tensor_tensor(out=t3[:], in0=t2[:], in1=t1[:],
                                        op=OP.subtract)
                nc.scalar.activation(out=sq[:], in_=t3[:], func=AF.Sqrt,
                                     bias=eps4k[:])
                nc.vector.reciprocal(out=r[:], in_=sq[:])
                nc.scalar.mul(r8[:], r[:], 64.0)
                nc.vector.tensor_tensor(out=sr[:], in0=S[:], in1=r[:], op=OP.mult)
                nc.vector.tensor_scalar(out=nb[:], in0=sr[:], scalar1=-1.0,
                                        scalar2=None, op0=OP.mult)
                return r8, nb

            def apply_ln(raw, r8, nb, m, gt, bt):
                # dst = raw * r8[:,m,:] + nb[:,m,:]  (broadcast along D)
                tmp = ntp.tile([P, NCH], bf16, tag="tmp", bufs=3)
                nt = ntp.tile([P, NCH], bf16, tag="nt", bufs=3)
                t3v = lambda t: t[:].rearrange("p (h d) -> p h d", h=HC)
                nc.vector.tensor_tensor(out=t3v(tmp), in0=t3v(raw),
                                        in1=bc_inner(r8[:, m, :]), op=OP.mult)
                if affine:
                    nt2 = ntp.tile([P, NCH], bf16, tag="nt2", bufs=3)
                    nc.vector.tensor_tensor(out=t3v(nt2), in0=t3v(tmp),
                                            in1=bc_inner(nb[:, m, :]), op=OP.add)
                    nc.vector.tensor_tensor(out=t3v(nt), in0=t3v(nt2),
                                            in1=bc_mid(gt[:], HC, D), op=OP.mult)
                    nc.vector.tensor_tensor(out=t3v(nt), in0=t3v(nt),
                                            in1=bc_mid(bt[:], HC, D), op=OP.add)
                else:
                    nc.vector.tensor_tensor(out=t3v(nt), in0=t3v(tmp),
                                            in1=bc_inner(nb[:, m, :]), op=OP.add)
                return nt

            for g in range(NG):
                wq = load_wch("q", g)
                wk = load_wch("k", g)
                wv = load_wch("v", g)

                def group8(kind, g, m0, wch, bias_off, xblk, dstT, gt, bt):
                    S = stp.tile([P, MQ, HC], f32, name=f"S_{kind}{g}_{m0}",
                                 tag="Ssum", bufs=2)
                    Q = stp.tile([P, MQ, HC], f32, name=f"Q_{kind}{g}_{m0}",
                                 tag="Qsum", bufs=2)
                    raws = []
                    for i in range(MQ):
                        m = m0 + i
                        xm = xs.tile([P, KB, P], bf16,
                                     name=f"x{kind}{g}_{m}", tag="x", bufs=2)
                        nc.sync.dma_start(out=xm[:], in_=xblk[m])
                        pt = qkv_mm(xm, wch, bias_off, f"pt{kind}{g}_{m}")
                        pt3 = pt[:].rearrange("p (h d) -> p h d", h=HC)
                        nc.vector.tensor_reduce(out=S[:, i, :], in_=pt3,
                                                axis=mybir.AxisListType.X,
                                                op=OP.add)
                        sqv = ntp.tile([P, NCH], f32, name=f"sq{kind}{g}_{m}",
                                       tag="sqv", bufs=2)
                        nc.scalar.activation(out=sqv[:], in_=pt[:],
                                             func=AF.Square)
                        nc.vector.tensor_reduce(
                            out=Q[:, i, :],
                            in_=sqv[:].rearrange("p (h d) -> p h d", h=HC),
                            axis=mybir.AxisListType.X, op=OP.add)
                        raw = rawp.tile([P, NCH], bf16, name=f"r{kind}{g}_{m}",
                                        tag="raw8", bufs=MQ)
                        nc.scalar.activation(out=raw[:], in_=pt[:],
                                             func=AF.Identity)
                        raws.append(raw)
                        if kind == "k":
                            pv = qkv_mm(xm, wv, 2 * E + g * NCH, f"ptv{g}_{m}")
                            nc.scalar.activation(
                                out=va_all[:, m, g * HC:(g + 1) * HC, 0:D],
                                in_=pv[:].rearrange("p (h d) -> p h d", h=HC),
                                func=AF.Identity)
                    r8, nb = chain(S, Q, MQ, f"{kind}{g}_{m0}")
                    for i in range(MQ):
                        nt = apply_ln(raws[i], r8, nb, i, gt, bt)
                        nc.sync.dma_start_transpose(
                            out=dstT[:, 4 * g:4 * g + 4,
                                     (m0 + i) * P:(m0 + i + 1) * P],
                            in_=nt[:])

                group8("q", g, 0, wq, g * NCH, xq_blk, qT_all,
                       gq_t if affine else None, bq_t if affine else None)
                for m0 in (0, MQ):
                    group8("k", g, m0, wk, E + g * NCH, xkv_blk, kT_all,
                           gk_t if affine else None, bk_t if affine else None)

        # -------- phase C: attention, software-pipelined unit stream --------
        with contextlib.ExitStack() as pc:
            pp = pc.enter_context(tc.tile_pool(name="pp", bufs=1))
            zp = pc.enter_context(tc.tile_pool(name="zp", bufs=1))

            NU = (H // 2) * MKV
            py_tiles = {}

            def emit_scores(u):
                j, tkb = divmod(u, MKV)
                sA = ps.tile([P, TQ], f32, name=f"sA_{u}", tag="scr2", bufs=2)
                sB = ps.tile([P, TQ], f32, name=f"sB_{u}", tag="scr2", bufs=2)
                for nk in range(TQ // NCH):
                    nsl = slice(nk * NCH, (nk + 1) * NCH)
                    nc.tensor.matmul(
                        sA[:, nsl],
                        kT_all[0:D, j, tkb * P:(tkb + 1) * P],
                        qT_all[0:D, j, nsl],
                        start=True, stop=True, tile_position=(0, 0))
                    nc.tensor.matmul(
                        sB[:, nsl],
                        kT_all[D:P, j, tkb * P:(tkb + 1) * P],
                        qT_all[D:P, j, nsl],
                        start=True, stop=True, tile_position=(64, 0))
                return sA, sB

            def emit_exp(u, s):
                sA, sB = s
                pA = pp.tile([P, TQ], bf16, name=f"pA_{u}", tag="p_bf", bufs=4)
                pB = pp.tile([P, TQ], bf16, name=f"pB_{u}", tag="p_bf", bufs=4)
                nc.scalar.activation(out=pA[:], in_=sA[:], func=AF.Exp,
                                     scale=SCALE)
                nc.scalar.activation(out=pB[:], in_=sB[:], func=AF.Exp,
                                     scale=SCALE)
                return pA, pB

            def emit_av(u, p):
                j, tkb = divmod(u, MKV)
                pA, pB = p
                if tkb == 0:
                    py_tiles[j] = (
                        ps.tile([P, TQ], f32, name=f"pyA_{j}", tag="py", bufs=2),
                        ps.tile([P, TQ], f32, name=f"pyB_{j}", tag="py", bufs=2))
                pyA, pyB = py_tiles[j]
                hA, hB = 2 * j, 2 * j + 1
                for nk in range(TQ // NCH):
                    nsl = slice(nk * NCH, (nk + 1) * NCH)
                    nc.tensor.matmul(pyA[0:D + 1, nsl], va_all[:, tkb, hA, :],
                                     pA[:, nsl],
                                     start=(tkb == 0), stop=(tkb == MKV - 1))
                    nc.tensor.matmul(pyB[0:D + 1, nsl], va_all[:, tkb, hB, :],
                                     pB[:, nsl],
                                     start=(tkb == 0), stop=(tkb == MKV - 1))

            def emit_tail(j):
                pyA, pyB = py_tiles.pop(j)
                for hh, py in ((0, pyA), (1, pyB)):
                    zst = zp.tile([1, TQ], f32, name=f"zst_{j}_{hh}",
                                  tag="zst", bufs=4)
                    nc.vector.tensor_copy(out=zst[:], in_=py[D:D + 1, :])
                    nc.gpsimd.dma_start(
                        out=z_sb[2 * j + hh:2 * j + hh + 1, :], in_=zst[:])
                nc.vector.tensor_copy(out=yT_raw[0:D, j, :], in_=pyA[0:D, :])
                nc.vector.tensor_copy(out=yT_raw[D:P, j, :], in_=pyB[0:D, :])

            def emit_norm():
                # batched 1/Z for all heads, bounce via DRAM, per-pair scale
                zr = zp.tile([H, TQ], f32, name="zr", tag="zr", bufs=1)
                nc.vector.reciprocal(out=zr[:], in_=z_sb[:])
                nc.sync.dma_start(out=z_dram[:], in_=zr[:])
                zt = z_dram[:]
                for j in range(H // 2):
                    zrep = zp.tile([P, TQ], bf16, name=f"zrep_{j}", tag="zrep",
                                   bufs=2)
                    for hh in range(2):
                        src = bass.AP(tensor=zt.tensor,
                                      offset=zt.offset + (2 * j + hh) * TQ,
                                      ap=[[0, D], [1, TQ]])
                        nc.gpsimd.dma_start(out=zrep[hh * D:(hh + 1) * D, :],
                                            in_=src)
                    nc.vector.tensor_tensor(out=yT_all[:, j, :],
                                            in0=yT_raw[:, j, :], in1=zrep[:],
                                            op=OP.mult)

            s_cur = emit_scores(0)
            av_p = None
            for u in range(NU):
                p_cur = emit_exp(u, s_cur)
                if av_p is not None:
                    emit_av(u - 1, av_p)
                    if (u - 1) % MKV == MKV - 1:
                        emit_tail((u - 1) // MKV)
                if u + 1 < NU:
                    s_cur = emit_scores(u + 1)
                av_p = p_cur
            emit_av(NU - 1, av_p)
            emit_tail(H // 2 - 1)
            emit_norm()

        # ---- phase D: output projection ----
        with contextlib.ExitStack() as pd:
            dwork = pd.enter_context(tc.tile_pool(name="dwork", bufs=1))
            for m in range(MQ):
                po = ps.tile([P, E], f32, name=f"po_{m}", tag="py", bufs=2)
                for nk in range(E // NCH):
                    nsl = slice(nk * NCH, (nk + 1) * NCH)
                    for kb in range(KB):
                        nc.tensor.matmul(po[:, nsl],
                                         yT_all[:, kb, m * P:(m + 1) * P],
                                         wp_all[:, kb, nsl],
                                         start=(kb == 0), stop=False)
                    nc.tensor.matmul(po[:, nsl], ones[0:1, :],
                                     bproj_row[0:1, nsl], start=False, stop=True)
                out_sb = dwork.tile([P, E], f32, tag="out_sb", bufs=2)
                nc.scalar.activation(out=out_sb[:], in_=po[:], func=AF.Identity)
                nc.sync.dma_start(out=out[m * P:(m + 1) * P, :], in_=out_sb[:])

    nc.finalize()
    return nc


def _get_nc(affine: bool):
    key = bool(affine)
    if key not in _BUILT:
        _BUILT[key] = _build_real(key)
    return _BUILT[key]


def kernel(x, Wqkv, bqkv, q_gamma, q_beta, k_gamma, k_beta, Wproj, bproj):
    from concourse.bass_utils import run_bass_kernel_spmd

    x = np.asarray(x, dtype=np.float32)
    Wqkv = np.asarray(Wqkv, dtype=np.float32)
    bqkv = np.asarray(bqkv, dtype=np.float32)
    Wproj = np.asarray(Wproj, dtype=np.float32)
    bproj = np.asarray(bproj, dtype=np.float32)
    q_gamma = np.asarray(q_gamma, dtype=np.float32)
    q_beta = np.asarray(q_beta, dtype=np.float32)
    k_gamma = np.asarray(k_gamma, dtype=np.float32)
    k_beta = np.asarray(k_beta, dtype=np.float32)

    affine = not (np.all(q_gamma == 1.0) and np.all(q_beta == 0.0)
                  and np.all(k_gamma == 1.0) and np.all(k_beta == 0.0))
    nc = _get_nc(affine)

    bf = ml_dtypes.bfloat16
    # Wblk[kind*2+g, p, kb, f] = Wqkv[kb*128+p, kind*E + g*512 + f]
    Wq3 = Wqkv.astype(bf).reshape(KB, P, 3, NG, NCH)  # kb, p, kind, g, f
    Wblk = np.ascontiguousarray(Wq3.transpose(2, 3, 1, 0, 4).reshape(
        3 * NG, P, KB, NCH))
    # Wpblk[p, kb, f] = Wproj[kb*128+p, f]
    Wpblk = np.ascontiguousarray(
        Wproj.astype(bf).reshape(KB, P, E).transpose(1, 0, 2))
    bqkv_b = bqkv.astype(bf)
    bproj_b = bproj.astype(bf)

    in_maps = []
    for c in range(NCORES):
        b, half = divmod(c, 2)
        xb = x[b].astype(bf)                       # [T, E]
        # xkv_blk[m, p, kb, t] = x[b][m*128+t, kb*128+p]
        xkv = np.ascontiguousarray(
            xb.reshape(MKV, P, KB, P).transpose(0, 3, 2, 1))
        xq = np.ascontiguousarray(xkv[half * MQ:(half + 1) * MQ])
        in_maps.append({
            "xq_blk": xq, "xkv_blk": xkv,
            "Wblk": Wblk, "bqkv": bqkv_b,
            "q_gamma": q_gamma, "q_beta": q_beta,
            "k_gamma": k_gamma, "k_beta": k_beta,
            "Wpblk": Wpblk, "bproj": bproj_b,
        })

    global _last_in_maps
    _last_in_maps = in_maps
    res = run_bass_kernel_spmd(nc, in_maps, core_ids=list(range(NCORES)))
    y = np.empty((B, T, E), dtype=np.float32)
    for c in range(NCORES):
        b, half = divmod(c, 2)
        y[b, half * TQ:(half + 1) * TQ, :] = res.results[c]["out"]
    return y


# revision 32
# speedup vs baseline: 1.3769x; 1.0153x over previous
# Multi-head attention block (QKV proj + per-head q/k layernorm + softmax
# attention + output proj) on 8 Trainium2 NeuronCores.
#
# Sharding: data-parallel over (batch, query-half). Core c handles batch
# c//2, query tokens [ (c%2)*1024, (c%2+1)*1024 ). Each core computes K/V
# for its batch's full 2048 tokens; no cross-core communication.
#
# Per-core dataflow (v2, engine-balanced + software-pipelined):
#   Phase A (QKV): host-pre-blocked xT / W tiles -> PE matmuls (N=512
#   chunks, q/k bias via K=1 ones-row matmul) -> grouped bn_stats (one per
#   512-wide tile) + per-group batched stats chain -> layernorm apply as two
#   broadcast tensor_tensor ops -> one wide XBAR DMA-transpose per tile into
#   feature-major qT/kT. V evicted via DVE add of a broadcast bias row.
#   Phase C (attention): software-pipelined unit stream (scores of unit u+1
#   and attn@V of unit u emitted around exp(u)) so ACT runs exp back-to-back
#   while PE stays dense; softmax normalizer Z rides as PSUM row 64 via a
#   ones-column in V; py evicted RAW + 1/Z kept per-j, normalization applied
#   later via one broadcast multiply per head-pair (Z broadcast via DRAM
#   bounce on the gpsimd queue, off the critical path).
#   Phase D: output projection from feature-major yT, bias via ones-row MM.
import contextlib

import numpy as np
import ml_dtypes

B, T, E = 4, 2048, 1024
H, D = 16, 64
P = 128
EPS = 1e-5
SCALE = 0.125  # 1/sqrt(D)
TQ = T // 2          # query tokens per core
KB = E // P          # contraction blocks
MQ = TQ // P         # query token tiles
MKV = T // P         # kv token tiles
FT = E // P          # feature tiles (qT/kT/yT)
NCORES = 8
HC = 8               # heads per 512-wide feature chunk
NCH = 512
NG = 2               # feature chunks

_BUILT = {}
_last_in_maps = None


def _build_real(affine: bool):
    import concourse.bass as bass
    import concourse.bacc as bacc
    import concourse.tile as tile
    from concourse import mybir

    f32 = mybir.dt.float32
    bf16 = mybir.dt.bfloat16
    AF = mybir.ActivationFunctionType
    OP = mybir.AluOpType

    nc = bacc.Bacc("TRN2", target_bir_lowering=False)
    xq_blk = nc.declare_dram_parameter("xq_blk", [MQ, P, KB, P], bf16, isOutput=False)
    xkv_blk = nc.declare_dram_parameter("xkv_blk", [MKV, P, KB, P], bf16, isOutput=False)
    Wblk = nc.declare_dram_parameter("Wblk", [3 * NG, P, KB, NCH], bf16, isOutput=False)
    bqkv = nc.declare_dram_parameter("bqkv", [3 * E], bf16, isOutput=False)
    q_gamma = nc.declare_dram_parameter("q_gamma", [D], f32, isOutput=False)
    q_beta = nc.declare_dram_parameter("q_beta", [D], f32, isOutput=False)
    k_gamma = nc.declare_dram_parameter("k_gamma", [D], f32, isOutput=False)
    k_beta = nc.declare_dram_parameter("k_beta", [D], f32, isOutput=False)
    Wpblk = nc.declare_dram_parameter("Wpblk", [P, KB, E], bf16, isOutput=False)
    bproj = nc.declare_dram_parameter("bproj", [E], bf16, isOutput=False)
    out = nc.declare_dram_parameter("out", [TQ, E], f32, isOutput=True)

    def bcast(dst, tensor, off, nparts, n):
        ap = bass.AP(tensor=tensor, offset=off, ap=[[0, nparts], [1, n]])
        nc.gpsimd.dma_start(out=dst, in_=ap)

    with tile.TileContext(nc) as tc, contextlib.ExitStack() as top:
        const = top.enter_context(tc.tile_pool(name="const", bufs=1))
        persist = top.enter_context(tc.tile_pool(name="persist", bufs=1))
        dr = top.enter_context(tc.tile_pool(name="dr", bufs=1, space="DRAM"))

        ones = const.tile([P, P], bf16)
        nc.vector.memset(ones[:], 1.0)
        eps4k = const.tile([P, 1], f32)
        nc.vector.memset(eps4k[:], 4096.0 * EPS)
        bqk_row = const.tile([1, 3 * E], bf16)
        nc.sync.dma_start(out=bqk_row[:], in_=bqkv[:])
        bproj_row = const.tile([1, E], bf16)
        nc.sync.dma_start(out=bproj_row[:], in_=bproj[:])
        wp_all = const.tile([P, KB, E], bf16)
        nc.sync.dma_start(out=wp_all[:], in_=Wpblk[:])
        if affine:
            gq_t = const.tile([P, D], f32)
            bq_t = const.tile([P, D], f32)
            gk_t = const.tile([P, D], f32)
            bk_t = const.tile([P, D], f32)
            bcast(gq_t[:], q_gamma, 0, P, D)
            bcast(bq_t[:], q_beta, 0, P, D)
            bcast(gk_t[:], k_gamma, 0, P, D)
            bcast(bk_t[:], k_beta, 0, P, D)

        va_all = persist.tile([P, MKV, H, D + 1], bf16)
        nc.vector.memset(va_all[:, :, :, D], 1.0)
        qT_all = persist.tile([P, FT, TQ], bf16)
        kT_all = persist.tile([P, FT, T], bf16)
        yT_raw = persist.tile([P, FT, TQ], bf16)
        yT_all = persist.tile([P, FT, TQ], bf16)
        z_sb = [persist.tile([HC, TQ], f32, name=f"z_sb{i}") for i in range(2)]
        z_dram = dr.tile([H, TQ], f32)

        def pe_filler(n):
            # dependency-free LDWEIGHTS to keep the PE HAM window busy
            for i in range(n):
                nc.tensor.ldweights(ones[:, 0:P])

        def bc_mid(t2d, nmid, ninner):
            # [P, ninner] AP viewed as [P, nmid(bcast), ninner]
            return bass.AP(tensor=t2d.tensor, offset=t2d.offset,
                           ap=[t2d.ap[0], [0, nmid], [1, ninner]])

        def bc_inner(t2d):
            # [P, HC] AP viewed as [P, h(8), d(64 bcast)]
            return bass.AP(tensor=t2d.tensor, offset=t2d.offset,
                           ap=[t2d.ap[0], [1, HC], [0, D]])

        # ---------------- phase A: QKV + LN + transposes ----------------
        with contextlib.ExitStack() as pa:
            wchp = pa.enter_context(tc.tile_pool(name="wchp", bufs=1))
            xs = pa.enter_context(tc.tile_pool(name="xs", bufs=1))
            rawp = pa.enter_context(tc.tile_pool(name="rawp", bufs=1))
            stp = pa.enter_context(tc.tile_pool(name="stp", bufs=1))
            ntp = pa.enter_context(tc.tile_pool(name="ntp", bufs=1))
            psA = pa.enter_context(tc.tile_pool(name="psA", bufs=1,
                                                space="PSUM"))

            def load_wch(kind, g):
                wch = wchp.tile([P, KB, NCH], bf16, name=f"w_{kind}{g}",
                                tag="wch", bufs=3)
                idx = {"q": 0, "k": 2, "v": 4}[kind] + g
                nc.sync.dma_start(out=wch[:], in_=Wblk[idx])
                return wch

            def qkv_mm(xm, wch, bias_off, name):
                pt = psA.tile([P, NCH], f32, name=name, tag="aps", bufs=6)
                for kb in range(KB):
                    nc.tensor.matmul(pt[:], xm[:, kb, :], wch[:, kb, :],
                                     start=(kb == 0), stop=(bias_off is None
                                                            and kb == KB - 1))
                if bias_off is not None:
                    nc.tensor.matmul(pt[:], ones[0:1, :],
                                     bqk_row[0:1, bias_off:bias_off + NCH],
                                     start=False, stop=True)
                return pt

            def chain(S, Q, nt_, uniq):
                # S=sum(x), Q=sum(x^2) per [P, nt, HC] ->
                #   rstd = 64/sqrt(64Q - S^2 + 4096 eps), nb = -S/sqrt(...)
                w = lambda nm: stp.tile([P, nt_, HC], f32,
                                        name=f"{uniq}_{nm}",
                                        tag=f"ch_{nm}", bufs=1)
                t1, t2, t3, sq, r, sr = (w(n) for n in
                                         ("t1", "t2", "t3", "sq", "r", "sr"))
                r8 = stp.tile([P, nt_, HC], f32, name=f"{uniq}_r8",
                              tag="ch_r8", bufs=1)
                nb = stp.tile([P, nt_, HC], f32, name=f"{uniq}_nb",
                              tag="ch_nb", bufs=1)
                nc.vector.tensor_tensor(out=t1[:], in0=S[:], in1=S[:], op=OP.mult)
                nc.scalar.mul(t2[:], Q[:], 64.0)
                nc.vector.tensor_tensor(out=t3[:], in0=t2[:], in1=t1[:],
                                        op=OP.subtract)
                nc.scalar.activation(out=sq[:], in_=t3[:], func=AF.Sqrt,
                                     bias=eps4k[:])
                nc.vector.reciprocal(out=r[:], in_=sq[:])
                nc.scalar.mul(r8[:], r[:], 64.0)
                nc.vector.tensor_tensor(out=sr[:], in0=S[:], in1=r[:], op=OP.mult)
                nc.vector.tensor_scalar(out=nb[:], in0=sr[:], scalar1=-1.0,
                                        scalar2=None, op0=OP.mult)
                return r8, nb

            def apply_ln(raw, r8, nb, m, gt, bt):
                # dst = raw * r8[:,m,:] + nb[:,m,:]  (broadcast along D)
                tmp = ntp.tile([P, NCH], bf16, tag="tmp", bufs=3)
                nt = ntp.tile([P, NCH], bf16, tag="nt", bufs=3)
                t3v = lambda t: t[:].rearrange("p (h d) -> p h d", h=HC)
                nc.vector.tensor_tensor(out=t3v(tmp), in0=t3v(raw),
                                        in1=bc_inner(r8[:, m, :]), op=OP.mult)
                if affine:
                    nt2 = ntp.tile([P, NCH], bf16, tag="nt2", bufs=3)
                    nc.vector.tensor_tensor(out=t3v(nt2), in0=t3v(tmp),
                                            in1=bc_inner(nb[:, m, :]), op=OP.add)
                    nc.vector.tensor_tensor(out=t3v(nt), in0=t3v(nt2),
                                            in1=bc_mid(gt[:], HC, D), op=OP.mult)
                    nc.vector.tensor_tensor(out=t3v(nt), in0=t3v(nt),
                                            in1=bc_mid(bt[:], HC, D), op=OP.add)
                else:
                    nc.vector.tensor_tensor(out=t3v(nt), in0=t3v(tmp),
                                            in1=bc_inner(nb[:, m, :]), op=OP.add)
                return nt

            for g in range(NG):
                wq = load_wch("q", g)
                wk = load_wch("k", g)
                wv = load_wch("v", g)

                def group8(kind, g, m0, wch, bias_off, xblk, dstT, gt, bt):
                    S = stp.tile([P, MQ, HC], f32, name=f"S_{kind}{g}_{m0}",
                                 tag="Ssum", bufs=2)
                    Q = stp.tile([P, MQ, HC], f32, name=f"Q_{kind}{g}_{m0}",
                                 tag="Qsum", bufs=2)
                    raws = []
                    for i in range(MQ):
                        m = m0 + i
                        xm = xs.tile([P, KB, P], bf16,
                                     name=f"x{kind}{g}_{m}", tag="x", bufs=2)
                        nc.sync.dma_start(out=xm[:], in_=xblk[m])
                        pt = qkv_mm(xm, wch, bias_off, f"pt{kind}{g}_{m}")
                        pt3 = pt[:].rearrange("p (h d) -> p h d", h=HC)
                        nc.vector.tensor_reduce(out=S[:, i, :], in_=pt3,
                                                axis=mybir.AxisListType.X,
                                                op=OP.add)
                        sqv = ntp.tile([P, NCH], f32, name=f"sq{kind}{g}_{m}",
                                       tag="sqv", bufs=2)
                        nc.scalar.activation(out=sqv[:], in_=pt[:],
                                             func=AF.Square)
                        nc.vector.tensor_reduce(
                            out=Q[:, i, :],
                            in_=sqv[:].rearrange("p (h d) -> p h d", h=HC),
                            axis=mybir.AxisListType.X, op=OP.add)
                        raw = rawp.tile([P, NCH], bf16, name=f"r{kind}{g}_{m}",
                                        tag="raw8", bufs=MQ)
                        nc.scalar.activation(out=raw[:], in_=pt[:],
                                             func=AF.Identity)
                        raws.append(raw)
                        if kind == "k":
                            pv = qkv_mm(xm, wv, 2 * E + g * NCH, f"ptv{g}_{m}")
                            nc.scalar.activation(
                                out=va_all[:, m, g * HC:(g + 1) * HC, 0:D],
                                in_=pv[:].rearrange("p (h d) -> p h d", h=HC),
                                func=AF.Identity)
                    r8, nb = chain(S, Q, MQ, f"{kind}{g}_{m0}")
                    for i in range(MQ):
                        nt = apply_ln(raws[i], r8, nb, i, gt, bt)
                        nc.sync.dma_start_transpose(
                            out=dstT[:, 4 * g:4 * g + 4,
                                     (m0 + i) * P:(m0 + i + 1) * P],
                            in_=nt[:])

                group8("q", g, 0, wq, g * NCH, xq_blk, qT_all,
                       gq_t if affine else None, bq_t if affine else None)
                for m0 in (0, MQ):
                    group8("k", g, m0, wk, E + g * NCH, xkv_blk, kT_all,
                           gk_t if affine else None, bk_t if affine else None)

        # -------- phase C: attention, software-pipelined unit stream --------
        with contextlib.ExitStack() as pc:
            pp = pc.enter_context(tc.tile_pool(name="pp", bufs=1))
            zp = pc.enter_context(tc.tile_pool(name="zp", bufs=1))
            ps = pc.enter_context(tc.tile_pool(name="psC", bufs=1,
                                               space="PSUM"))

            NU = (H // 2) * MKV
            py_tiles = {}

            def emit_scores(u):
                j, tkb = divmod(u, MKV)
                sA = ps.tile([P, TQ], f32, name=f"sA_{u}", tag="scr2", bufs=2)
                sB = ps.tile([P, TQ], f32, name=f"sB_{u}", tag="scr2", bufs=2)
                for nk in range(TQ // NCH):
                    nsl = slice(nk * NCH, (nk + 1) * NCH)
                    nc.tensor.matmul(
                        sA[:, nsl],
                        kT_all[0:D, j, tkb * P:(tkb + 1) * P],
                        qT_all[0:D, j, nsl],
                        start=True, stop=True, tile_position=(0, 0))
                    nc.tensor.matmul(
                        sB[:, nsl],
                        kT_all[D:P, j, tkb * P:(tkb + 1) * P],
                        qT_all[D:P, j, nsl],
                        start=True, stop=True, tile_position=(64, 0))
                return sA, sB

            def emit_exp(u, s):
                sA, sB = s
                pA = pp.tile([P, TQ], bf16, name=f"pA_{u}", tag="p_bf", bufs=4)
                pB = pp.tile([P, TQ], bf16, name=f"pB_{u}", tag="p_bf", bufs=4)
                nc.scalar.activation(out=pA[:], in_=sA[:], func=AF.Exp,
                                     scale=SCALE)
                nc.scalar.activation(out=pB[:], in_=sB[:], func=AF.Exp,
                                     scale=SCALE)
                return pA, pB

            def emit_av(u, p):
                j, tkb = divmod(u, MKV)
                pA, pB = p
                if tkb == 0:
                    py_tiles[j] = (
                        ps.tile([P, TQ], f32, name=f"pyA_{j}", tag="py", bufs=2),
                        ps.tile([P, TQ], f32, name=f"pyB_{j}", tag="py", bufs=2))
                pyA, pyB = py_tiles[j]
                hA, hB = 2 * j, 2 * j + 1
                for nk in range(TQ // NCH):
                    nsl = slice(nk * NCH, (nk + 1) * NCH)
                    nc.tensor.matmul(pyA[0:D + 1, nsl], va_all[:, tkb, hA, :],
                                     pA[:, nsl],
                                     start=(tkb == 0), stop=(tkb == MKV - 1))
                    nc.tensor.matmul(pyB[0:D + 1, nsl], va_all[:, tkb, hB, :],
                                     pB[:, nsl],
                                     start=(tkb == 0), stop=(tkb == MKV - 1))

            def emit_tail(j):
                pyA, pyB = py_tiles.pop(j)
                for hh, py in ((0, pyA), (1, pyB)):
                    zst = zp.tile([1, TQ], f32, name=f"zst_{j}_{hh}",
                                  tag="zst", bufs=4)
                    nc.vector.tensor_copy(out=zst[:], in_=py[D:D + 1, :])
                    half, row = divmod(2 * j + hh, HC)
                    nc.gpsimd.dma_start(
                        out=z_sb[half][row:row + 1, :], in_=zst[:])
                nc.vector.tensor_copy(out=yT_raw[0:D, j, :], in_=pyA[0:D, :])
                nc.vector.tensor_copy(out=yT_raw[D:P, j, :], in_=pyB[0:D, :])

            def emit_norm(half):
                # batched 1/Z for 8 heads, bounce via DRAM, per-pair scale
                zr = zp.tile([HC, TQ], f32, name=f"zr{half}", tag="zr", bufs=2)
                nc.vector.reciprocal(out=zr[:], in_=z_sb[half][:])
                nc.sync.dma_start(out=z_dram[half * HC:(half + 1) * HC, :],
                                  in_=zr[:])
                zt = z_dram[:]
                for j in range(half * 4, half * 4 + 4):
                    zrep = zp.tile([P, TQ], bf16, name=f"zrep_{j}", tag="zrep",
                                   bufs=2)
                    for hh in range(2):
                        src = bass.AP(tensor=zt.tensor,
                                      offset=zt.offset + (2 * j + hh) * TQ,
                                      ap=[[0, D], [1, TQ]])
                        nc.gpsimd.dma_start(out=zrep[hh * D:(hh + 1) * D, :],
                                            in_=src)
                    nc.vector.tensor_tensor(out=yT_all[:, j, :],
                                            in0=yT_raw[:, j, :], in1=zrep[:],
                                            op=OP.mult)

            s_cur = emit_scores(0)
            av_p = None
            for u in range(NU):
                p_cur = emit_exp(u, s_cur)
                pe_filler(8)
                if av_p is not None:
                    emit_av(u - 1, av_p)
                    if (u - 1) % MKV == MKV - 1:
                        jj = (u - 1) // MKV
                        emit_tail(jj)
                        if jj == 3:
                            emit_norm(0)
                if u + 1 < NU:
                    s_cur = emit_scores(u + 1)
                av_p = p_cur
            emit_av(NU - 1, av_p)
            emit_tail(H // 2 - 1)
            emit_norm(1)

        # ---- phase D: output projection ----
        with contextlib.ExitStack() as pd:
            dwork = pd.enter_context(tc.tile_pool(name="dwork", bufs=1))
            psD = pd.enter_context(tc.tile_pool(name="psD", bufs=1,
                                                space="PSUM"))
            for m in range(MQ):
                po = psD.tile([P, E], f32, name=f"po_{m}", tag="po", bufs=2)
                for nk in range(E // NCH):
                    nsl = slice(nk * NCH, (nk + 1) * NCH)
                    for kb in range(KB):
                        nc.tensor.matmul(po[:, nsl],
                                         yT_all[:, kb, m * P:(m + 1) * P],
                                         wp_all[:, kb, nsl],
                                         start=(kb == 0), stop=False)
                    nc.tensor.matmul(po[:, nsl], ones[0:1, :],
                                     bproj_row[0:1, nsl], start=False, stop=True)
                out_sb = dwork.tile([P, E], f32, tag="out_sb", bufs=2)
                nc.scalar.activation(out=out_sb[:], in_=po[:], func=AF.Identity)
                nc.sync.dma_start(out=out[m * P:(m + 1) * P, :], in_=out_sb[:])

    nc.finalize()
    return nc


def _get_nc(affine: bool):
    key = bool(affine)
    if key not in _BUILT:
        _BUILT[key] = _build_real(key)
    return _BUILT[key]


def kernel(x, Wqkv, bqkv, q_gamma, q_beta, k_gamma, k_beta, Wproj, bproj):
    from concourse.bass_utils import run_bass_kernel_spmd

    x = np.asarray(x, dtype=np.float32)
    Wqkv = np.asarray(Wqkv, dtype=np.float32)
    bqkv = np.asarray(bqkv, dtype=np.float32)
    Wproj = np.asarray(Wproj, dtype=np.float32)
    bproj = np.asarray(bproj, dtype=np.float32)
    q_gamma = np.asarray(q_gamma, dtype=np.float32)
    q_beta = np.asarray(q_beta, dtype=np.float32)
    k_gamma = np.asarray(k_gamma, dtype=np.float32)
    k_beta = np.asarray(k_beta, dtype=np.float32)

    affine = not (np.all(q_gamma == 1.0) and np.all(q_beta == 0.0)
                  and np.all(k_gamma == 1.0) and np.all(k_beta == 0.0))
    nc = _get_nc(affine)

    bf = ml_dtypes.bfloat16
    # Wblk[kind*2+g, p, kb, f] = Wqkv[kb*128+p, kind*E + g*512 + f]
    Wq3 = Wqkv.astype(bf).reshape(KB, P, 3, NG, NCH)  # kb, p, kind, g, f
    Wblk = np.ascontiguousarray(Wq3.transpose(2, 3, 1, 0, 4).reshape(
        3 * NG, P, KB, NCH))
    # Wpblk[p, kb, f] = Wproj[kb*128+p, f]
    Wpblk = np.ascontiguousarray(
        Wproj.astype(bf).reshape(KB, P, E).transpose(1, 0, 2))
    bqkv_b = bqkv.astype(bf)
    bproj_b = bproj.astype(bf)

    in_maps = []
    for c in range(NCORES):
        b, half = divmod(c, 2)
        xb = x[b].astype(bf)                       # [T, E]
        # xkv_blk[m, p, kb, t] = x[b][m*128+t, kb*128+p]
        xkv = np.ascontiguousarray(
            xb.reshape(MKV, P, KB, P).transpose(0, 3, 2, 1))
        xq = np.ascontiguousarray(xkv[half * MQ:(half + 1) * MQ])
        in_maps.append({
            "xq_blk": xq, "xkv_blk": xkv,
            "Wblk": Wblk, "bqkv": bqkv_b,
            "q_gamma": q_gamma, "q_beta": q_beta,
            "k_gamma": k_gamma, "k_beta": k_beta,
            "Wpblk": Wpblk, "bproj": bproj_b,
        })

    global _last_in_maps
    _last_in_maps = in_maps
    res = run_bass_kernel_spmd(nc, in_maps, core_ids=list(range(NCORES)))
    y = np.empty((B, T, E), dtype=np.float32)
    for c in range(NCORES):
        b, half = divmod(c, 2)
        y[b, half * TQ:(half + 1) * TQ, :] = res.results[c]["out"]
    return y
